# revision 4
# baseline (speedup 1.0000x reference)
r"""Trainium2 Bass kernel for nn_Attention (B=32, P=577, D=768, 12 heads).

Data-parallel over batch: 4 batch elements per core on 8 cores, zero
collectives. Transposed dataflow ([feature, token]) so every matmul
consumes the previous stage's output with no on-chip transposes:

  xT [768,577/b] --(wqkvT)--> qkT [1536,577] (features on partitions)
                         \--> V   [577,768]  (tokens on partitions)
  per (b,h): ST = K Q^T  (K=64 head pairs in PE row halves)
             AT = exp(scale*ST)  (ACT; |scale*S| small, no max-subtract)
  O_u^T[65,q] = [V|1]^T AT   (ones column -> softmax denominators row 64)
  DVE copies O_u^T PSUM->SBUF (frees the PSUM slot fast), then
  rec = 1/denom (DVE), bcast (Pool), O^T = O_u^T * rec (DVE)
  yT [768,577] = w_outT^T O^T + b_eff   (bf16 out DMA; host casts f32)

Schedule: per batch b, attention j2-blocks interleave at fine grain with
proj(b+1) and deferred outproj chains; outproj is deferred up to two
batches to bank PE work for the ACT-bound final-batch window.

PSUM (8 banks): ring "st" 2x[128,577] (4 banks) serves ST tiles AND av
accumulators; ring "g577" 2x[128,577] (4 banks) serves proj/outproj.
This keeps proj(b+1) PSUM independent of attn(b)'s normalization chain.

Engine discipline: ACT runs ONLY Exp (a mixed ACT stream risks
activation-table reloads on real HW, ~1.3us each); generic gpsimd tensor
ops avoided (Q7 launch/ucode cost on HW far exceeds the cost model);
Pool does only partition_broadcast.
"""

import numpy as np
import ml_dtypes

import concourse.bass as bass
import concourse.tile as tile
from concourse import bacc, mybir
from concourse.bass_utils import run_bass_kernel_spmd

B, PL, D = 32, 577, 768
H, S = 12, 64
NCORES = 8
NB = B // NCORES          # 4 batches per core
T = NB * PL               # 2308 tokens per core
P = 128
DT = D // P               # 6 contraction tiles
SCALE = float((D // 8) ** -0.5)   # 96**-0.5 (module bug kept faithful)

FB = mybir.dt.bfloat16
F32 = mybir.dt.float32

QS = [128, 128, 128, 128, 65]          # q-subtiles of 577
PCH = [(0, 512), (512, 65)]            # p-chunks of 577 (PSUM bank = 512 f32)
VCH = [(0, 512), (512, 256)]           # V projection chunks of 768


def build_bass(reps=1):
    nc = bacc.Bacc("TRN2", target_bir_lowering=False, debug=False,
                   num_devices=NCORES)

    x_t = nc.dram_tensor("x_t", [D, T], FB, kind="ExternalInput").ap()
    w_qkv_t = nc.dram_tensor("w_qkv_t", [D, 3 * D], FB, kind="ExternalInput").ap()
    w_out_t = nc.dram_tensor("w_out_t", [D, D], FB, kind="ExternalInput").ap()
    b_qk = nc.dram_tensor("b_qk", [P, 12], F32, kind="ExternalInput").ap()
    b_out = nc.dram_tensor("b_out", [P, DT], F32, kind="ExternalInput").ap()
    out_d = nc.dram_tensor("out", [D, T], FB, kind="ExternalOutput").ap()
    out_v = out_d.rearrange("(o p) t -> p o t", p=P)
    xv = x_t.rearrange("(o p) t -> p o t", p=P)
    wv = w_qkv_t.rearrange("(o p) e -> p o e", p=P)

    with tile.TileContext(nc) as tc:
      for _rep in range(reps):  # >1 only for differential benchmarking
        with tc.tile_pool(name="singles", bufs=1) as singles, \
             tc.tile_pool(name="bt", bufs=2) as btp, \
             tc.tile_pool(name="otp", bufs=4) as otp, \
             tc.tile_pool(name="atp", bufs=4) as atpool, \
             tc.tile_pool(name="nrm", bufs=4) as nrm, \
             tc.tile_pool(name="avs", bufs=4) as avsp, \
             tc.tile_pool(name="yout", bufs=4) as ypool, \
             tc.tile_pool(name="pst", bufs=2, space="PSUM") as pst, \
             tc.tile_pool(name="pg", bufs=2, space="PSUM") as pg:
            # ---- input DMAs: interleave x(b0) and wv k-tiles so the V
            # projection (cheapest full dependency set) starts ~2us in; then
            # wqk for the QK chains; then the rest of x merged per k.
            xt = {}
            wvp, wqk = [], []
            for k in range(DT):
                xk = singles.tile([P, PL], FB, tag=f"x{k}_0", name=f"x{k}_0")
                nc.sync.dma_start(xk[:], xv[:, k, 0:PL])
                xt[(k, 0)] = xk
                wk = singles.tile([P, D], FB, tag=f"wv{k}", name=f"wv{k}")
                nc.sync.dma_start(wk[:], wv[:, k, 0:D])
                wvp.append(wk)
            for k in range(DT):
                wk = singles.tile([P, 2 * D], FB, tag=f"wqk{k}",
                                  name=f"wqk{k}")
                nc.sync.dma_start(wk[:], wv[:, k, D:3 * D])
                wqk.append(wk)
            bqk = singles.tile([P, 12], F32, tag="bqk")
            nc.sync.dma_start(bqk[:], b_qk)
            for k in range(DT):
                xk = singles.tile([P, (NB - 1) * PL], FB, tag=f"x{k}_r",
                                  name=f"x{k}_r")
                nc.sync.dma_start(xk[:], xv[:, k, PL:T])
                for b in range(1, NB):
                    xt[(k, b)] = xk[:, (b - 1) * PL: b * PL]
            bo = singles.tile([P, DT], F32, tag="bo")
            nc.sync.dma_start(bo[:], b_out)
            wo = singles.tile([P, DT, D], FB, tag="wo")
            nc.sync.dma_start(wo[:], w_out_t.rearrange("(o p) e -> p o e", p=P))

            qkt, vbuf, ot = {}, {}, {}

            def emit_proj_qk(b, j):
                ps = pg.tile([P, PL], F32, tag="g577", name="psqk")
                for k in range(DT):
                    for (c0, cw) in PCH:
                        nc.tensor.matmul(
                            ps[:, c0:c0 + cw],
                            lhsT=wqk[k][:, j * P:(j + 1) * P],
                            rhs=xt[(k, b)][:, c0:c0 + cw],
                            start=(k == 0), stop=(k == DT - 1),
                            skip_group_check=True)
                qt_tile = btp.tile([P, PL], FB, tag=f"qkt{j}", name=f"qkt{j}")
                nc.vector.tensor_scalar_add(qt_tile[:], ps[:], bqk[:, j:j + 1])
                qkt[(b, j)] = qt_tile

            def emit_proj_v(b, tt):
                rows = QS[tt]
                ps = pg.tile([P, D], F32, tag="g577", name="psv")
                for k in range(DT):
                    for (c0, cw) in VCH:
                        nc.tensor.matmul(
                            ps[:rows, c0:c0 + cw],
                            lhsT=xt[(k, b)][:, tt * P: tt * P + rows],
                            rhs=wvp[k][:, c0:c0 + cw],
                            start=(k == 0), stop=(k == DT - 1),
                            skip_group_check=True)
                vt = btp.tile([P, H, S + 1], FB, tag=f"v{tt}", name=f"v{tt}")
                nc.vector.memset(vt[:, :, S:S + 1], 1.0)
                nc.vector.tensor_copy(
                    vt[:rows, :, 0:S],
                    ps[:rows].rearrange("p (h s) -> p h s", h=H))
                vbuf[(b, tt)] = vt

            def emit_proj(b):
                # V first: its inputs (x(b0)+wv, interleaved DMAs) land first
                for tt in range(5):
                    emit_proj_v(b, tt)
                for j in (0, 6, 1, 7, 2, 8, 3, 9, 4, 10, 5, 11):
                    emit_proj_qk(b, j)
                alloc_ot(b)

            def alloc_ot(b):
                for j in range(DT):
                    ot[(b, j)] = otp.tile([P, PL], FB, tag=f"ot{j}", name=f"ot{j}")

            def emit_av_norm(b, h, at, final=False):
                j2, hp = h // 2, (h % 2) * 64
                last = (b == NB - 1)
                # av shares the "st" ring (same shape); freed fast by the
                # copy-out so next-j2 STs resume quickly.
                av = pst.tile([P, PL], F32, tag="st", name="av")
                for qt in range(5):
                    rows = QS[qt]
                    for (c0, cw) in PCH:
                        nc.tensor.matmul(
                            av[0:S + 1, c0:c0 + cw],
                            lhsT=vbuf[(b, qt)][:rows, h, :],
                            rhs=at[:rows, qt, c0:c0 + cw],
                            start=(qt == 0), stop=(qt == 4),
                            skip_group_check=True)
                # copy PSUM->SBUF so the ring slot frees quickly. ACT has
                # slack in steady state; in the last batch (no proj work
                # left) ACT is the binding engine, so copy on DVE there.
                if final:
                    # very last chain: normalize straight from PSUM (the ring
                    # has no successors) — drops the copy from the critical
                    # path into the final out-projection.
                    rec = nrm.tile([1, PL], F32, tag="rec", name="rec")
                    nc.vector.reciprocal(rec[:], av[S:S + 1, :])
                    recb = nrm.tile([64, PL], F32, tag="recb", name="recb")
                    nc.gpsimd.partition_broadcast(recb[:], rec[:])
                    if hp == 0:
                        nc.vector.tensor_mul(
                            out=ot[(b, j2)][0:S, :], in0=av[0:S, :], in1=recb[:])
                    else:
                        tmp = nrm.tile([64, PL], FB, tag="tmp", name="tmp")
                        nc.vector.tensor_mul(out=tmp[:], in0=av[0:S, :],
                                             in1=recb[:])
                        nc.sync.dma_start(ot[(b, j2)][64:128, :], tmp[:])
                    return
                # copy on DVE: keeps ACT pure-Exp (a mixed ACT stream can
                # thrash activation-table loads on real hardware)
                avs = avsp.tile([S + 1, PL], F32, tag="avs", name="avs")
                nc.vector.tensor_copy(avs[:], av[0:S + 1, :])
                rec = nrm.tile([1, PL], F32, tag="rec", name="rec")
                nc.vector.reciprocal(rec[:], avs[S:S + 1, :])
                recb = nrm.tile([64, PL], F32, tag="recb", name="recb")
                nc.gpsimd.partition_broadcast(recb[:], rec[:])
                if hp == 0:
                    nc.vector.tensor_mul(
                        out=ot[(b, j2)][0:S, :], in0=avs[0:S, :], in1=recb[:])
                else:
                    tmp = nrm.tile([64, PL], FB, tag="tmp", name="tmp")
                    nc.vector.tensor_mul(out=tmp[:], in0=avs[0:S, :], in1=recb[:])
                    nc.sync.dma_start(ot[(b, j2)][64:128, :], tmp[:])

            def emit_attn_j2(b, j2):
                # head pairs in PE row halves (partitions 0-63 / 64-127):
                # their K=64 ST matmuls run in opposite PE row groups.
                h0 = 2 * j2
                qs_ = qkt[(b, j2)]
                ks_ = qkt[(b, 6 + j2)]
                at0 = atpool.tile([P, 5, PL], FB, tag="at", name="at0")
                at1 = atpool.tile([P, 5, PL], FB, tag="at", name="at1")
                for qt in range(5):
                    rows = QS[qt]
                    st0 = pst.tile([P, PL], F32, tag="st", name="st0")
                    st1 = pst.tile([P, PL], F32, tag="st", name="st1")
                    for (c0, cw) in PCH:
                        nc.tensor.matmul(
                            st0[:rows, c0:c0 + cw],
                            lhsT=ks_[0:64, qt * P: qt * P + rows],
                            rhs=qs_[0:64, c0:c0 + cw],
                            start=True, stop=True,
                            skip_group_check=True)
                        nc.tensor.matmul(
                            st1[:rows, c0:c0 + cw],
                            lhsT=ks_[64:128, qt * P: qt * P + rows],
                            rhs=qs_[64:128, c0:c0 + cw],
                            start=True, stop=True,
                            skip_group_check=True)
                    nc.scalar.activation(
                        at0[:rows, qt, :], st0[:rows, :],
                        mybir.ActivationFunctionType.Exp, scale=SCALE)
                    nc.scalar.activation(
                        at1[:rows, qt, :], st1[:rows, :],
                        mybir.ActivationFunctionType.Exp, scale=SCALE)
                fin = (b == NB - 1 and j2 == 5)
                if fin:
                    # odd head first: its SBUF->SBUF hop overlaps h0's chain
                    emit_av_norm(b, h0 + 1, at1, final=True)
                    emit_av_norm(b, h0, at0, final=True)
                else:
                    emit_av_norm(b, h0, at0)
                    emit_av_norm(b, h0 + 1, at1)

            def emit_outproj_m(b, m, pool=None):
                ps = (pool or pg).tile([P, PL], F32,
                                       tag="g577" if pool is None else "st",
                                       name="psy")
                for k in range(DT):
                    for (c0, cw) in PCH:
                        nc.tensor.matmul(
                            ps[:, c0:c0 + cw],
                            lhsT=wo[:, k, m * P:(m + 1) * P],
                            rhs=ot[(b, k)][:, c0:c0 + cw],
                            start=(k == 0), stop=(k == DT - 1),
                            skip_group_check=True)
                ysb = ypool.tile([P, PL], FB, tag="ysb", name="ysb")
                nc.vector.tensor_scalar_add(ysb[:], ps[:], bo[:, m:m + 1])
                nc.sync.dma_start(out_v[:, m, b * PL:(b + 1) * PL], ysb[:])

            # Pipeline: per batch b, interleave attention j2-blocks with the
            # next batch's projection and the previous batch's out-projection
            # at fine grain, so the shared "g577" ring alternates short-hold
            # allocations and every engine always has runnable work nearby.
            # outproj chains are deferred ~two batches: they are the only
            # freely-schedulable PE work, so bank them as a backlog that
            # fills the ACT-bound final-batch window.
            emit_proj(0)
            for b in range(NB):
                for j2 in range(6):
                    emit_attn_j2(b, j2)
                    if b + 1 < NB:
                        emit_proj_qk(b + 1, 2 * j2)
                        emit_proj_qk(b + 1, 2 * j2 + 1)
                        if j2 < 5:
                            emit_proj_v(b + 1, j2)
                    if b == 2 and j2 % 2 == 0:
                        emit_outproj_m(0, j2 // 2)
                    if b == 3:
                        # backlog drain: outproj(0) tail, (1), (2) interleave
                        if j2 < 3:
                            emit_outproj_m(0, 3 + j2)
                        emit_outproj_m(1, j2)
                        emit_outproj_m(2, j2)
                if b + 1 < NB:
                    alloc_ot(b + 1)
            # final outproj: alternate PSUM rings (st ring is free now)
            for m in range(DT):
                emit_outproj_m(NB - 1, m, pool=pst if m % 2 else None)

    nc.compile()
    return nc


_NC = None


def _get_nc():
    global _NC
    if _NC is None:
        _NC = build_bass()
    return _NC


def make_in_maps(x, qkv_w, qkv_b, out_w, out_b):
    """Host-side shard + layout prep. Returns per-core input dicts."""
    bf16 = ml_dtypes.bfloat16
    x = np.asarray(x, dtype=np.float32)
    qkv_w = np.asarray(qkv_w, dtype=np.float32)
    qkv_b = np.asarray(qkv_b, dtype=np.float32)
    out_w = np.asarray(out_w, dtype=np.float32)
    out_b = np.asarray(out_b, dtype=np.float32)

    w_qkv_t = np.ascontiguousarray(qkv_w.T).astype(bf16)          # [768, 2304]
    w_out_t = np.ascontiguousarray(out_w.T).astype(bf16)          # [768, 768]
    b_qk = np.ascontiguousarray(qkv_b[D:3 * D].reshape(12, P).T)  # [128, 12]
    # v-bias passes linearly through the output projection (softmax rows sum
    # to 1): fold it into an effective output bias.
    b_eff = out_b + out_w @ qkv_b[0:D]
    b_out = np.ascontiguousarray(b_eff.reshape(DT, P).T)          # [128, 6]

    in_maps = []
    for c in range(NCORES):
        xc = x[c * NB:(c + 1) * NB].reshape(T, D)                 # [2308, 768]
        x_t = np.ascontiguousarray(xc.T).astype(bf16)             # [768, 2308]
        in_maps.append({
            "x_t": x_t,
            "w_qkv_t": w_qkv_t,
            "w_out_t": w_out_t,
            "b_qk": b_qk.astype(np.float32),
            "b_out": b_out.astype(np.float32),
        })
    return in_maps


def assemble_output(results):
    """Per-core 'out' [768, 2308] bf16 -> full [32, 577, 768] f32."""
    y = np.empty((B, PL, D), dtype=np.float32)
    for c in range(NCORES):
        yt = results[c]["out"].astype(np.float32)                 # [768, 2308]
        y[c * NB:(c + 1) * NB] = yt.T.reshape(NB, PL, D)
    return y


def run(x, qkv_w, qkv_b, out_w, out_b, trace=False):
    nc = _get_nc()
    in_maps = make_in_maps(x, qkv_w, qkv_b, out_w, out_b)
    res = run_bass_kernel_spmd(nc, in_maps, core_ids=list(range(NCORES)),
                               trace=trace)
    return assemble_output(res.results), res


def kernel(x, qkv_w, qkv_b, out_w, out_b):
    y, _ = run(x, qkv_w, qkv_b, out_w, out_b)
    return y


# revision 5
# speedup vs baseline: 2.4476x; 2.4476x over previous
r"""Trainium2 Bass kernel for nn_Attention (B=32, P=577, D=768, 12 heads).

Data-parallel over batch: 4 batch elements per core on 8 cores, zero
collectives. Transposed dataflow ([feature, token]) so every matmul
consumes the previous stage's output with no on-chip transposes:

  xT [768,577/b] --(wqkvT)--> qkT [1536,577] (features on partitions)
                         \--> V   [577,768]  (tokens on partitions)
  per (b,h): ST = K Q^T  (K=64 head pairs in PE row halves)
             AT = exp(scale*ST)  (ACT; |scale*S| small, no max-subtract)
  O_u^T[65,q] = [V|1]^T AT   (ones column -> softmax denominators row 64)
  DVE copies O_u^T PSUM->SBUF (frees the PSUM slot fast), then
  rec = 1/denom (DVE), bcast (Pool), O^T = O_u^T * rec (DVE)
  yT [768,577] = w_outT^T O^T + b_eff   (bf16 out DMA; host casts f32)

Schedule: per batch b, attention j2-blocks interleave at fine grain with
proj(b+1) and deferred outproj chains; outproj is deferred up to two
batches to bank PE work for the ACT-bound final-batch window.

PSUM (8 banks): ring "st" 2x[128,577] (4 banks) serves ST tiles AND av
accumulators; ring "g577" 2x[128,577] (4 banks) serves proj/outproj.
This keeps proj(b+1) PSUM independent of attn(b)'s normalization chain.

Engine discipline: ACT runs ONLY Exp (a mixed ACT stream risks
activation-table reloads on real HW, ~1.3us each); generic gpsimd tensor
ops avoided (Q7 launch/ucode cost on HW far exceeds the cost model);
Pool does only partition_broadcast.
"""

import numpy as np
import ml_dtypes

import concourse.bass as bass
import concourse.tile as tile
from concourse import bacc, mybir
from concourse.bass_utils import run_bass_kernel_spmd

B, PL, D = 32, 577, 768
H, S = 12, 64
NCORES = 8
NB = B // NCORES          # 4 batches per core
T = NB * PL               # 2308 tokens per core
P = 128
DT = D // P               # 6 contraction tiles
SCALE = float((D // 8) ** -0.5)   # 96**-0.5 (module bug kept faithful)

FB = mybir.dt.bfloat16
F32 = mybir.dt.float32

QS = [128, 128, 128, 128, 65]          # q-subtiles of 577
PCH = [(0, 512), (512, 65)]            # p-chunks of 577 (PSUM bank = 512 f32)
VCH = [(0, 512), (512, 256)]           # V projection chunks of 768


def build_bass(reps=1):
    nc = bacc.Bacc("TRN2", target_bir_lowering=False, debug=False,
                   num_devices=NCORES)

    x_t = nc.dram_tensor("x_t", [D, T], FB, kind="ExternalInput").ap()
    w_qkv_t = nc.dram_tensor("w_qkv_t", [D, 3 * D], FB, kind="ExternalInput").ap()
    w_out_t = nc.dram_tensor("w_out_t", [D, D], FB, kind="ExternalInput").ap()
    b_qk = nc.dram_tensor("b_qk", [P, 12], F32, kind="ExternalInput").ap()
    b_out = nc.dram_tensor("b_out", [P, DT], F32, kind="ExternalInput").ap()
    out_d = nc.dram_tensor("out", [D, T], FB, kind="ExternalOutput").ap()
    out_v = out_d.rearrange("(o p) t -> p o t", p=P)
    xv = x_t.rearrange("(o p) t -> p o t", p=P)
    wv = w_qkv_t.rearrange("(o p) e -> p o e", p=P)

    with tile.TileContext(nc) as tc:
      for _rep in range(reps):  # >1 only for differential benchmarking
        with tc.tile_pool(name="singles", bufs=1) as singles, \
             tc.tile_pool(name="bt", bufs=2) as btp, \
             tc.tile_pool(name="otp", bufs=4) as otp, \
             tc.tile_pool(name="atp", bufs=4) as atpool, \
             tc.tile_pool(name="nrm", bufs=4) as nrm, \
             tc.tile_pool(name="avs", bufs=4) as avsp, \
             tc.tile_pool(name="yout", bufs=4) as ypool, \
             tc.tile_pool(name="pst", bufs=2, space="PSUM") as pst, \
             tc.tile_pool(name="pg", bufs=2, space="PSUM") as pg:
            # ---- input DMAs: interleave x(b0) and wv k-tiles so the V
            # projection (cheapest full dependency set) starts ~2us in; then
            # wqk for the QK chains; then the rest of x merged per k.
            xt = {}
            wvp, wqk = [], []
            for k in range(DT):
                xk = singles.tile([P, PL], FB, tag=f"x{k}_0", name=f"x{k}_0")
                nc.sync.dma_start(xk[:], xv[:, k, 0:PL])
                xt[(k, 0)] = xk
                wk = singles.tile([P, D], FB, tag=f"wv{k}", name=f"wv{k}")
                nc.sync.dma_start(wk[:], wv[:, k, 0:D])
                wvp.append(wk)
            for k in range(DT):
                wk = singles.tile([P, 2 * D], FB, tag=f"wqk{k}",
                                  name=f"wqk{k}")
                nc.sync.dma_start(wk[:], wv[:, k, D:3 * D])
                wqk.append(wk)
            bqk = singles.tile([P, 12], F32, tag="bqk")
            nc.sync.dma_start(bqk[:], b_qk)
            for k in range(DT):
                xk = singles.tile([P, (NB - 1) * PL], FB, tag=f"x{k}_r",
                                  name=f"x{k}_r")
                nc.sync.dma_start(xk[:], xv[:, k, PL:T])
                for b in range(1, NB):
                    xt[(k, b)] = xk[:, (b - 1) * PL: b * PL]
            bo = singles.tile([P, DT], F32, tag="bo")
            nc.sync.dma_start(bo[:], b_out)
            wo = singles.tile([P, DT, D], FB, tag="wo")
            nc.sync.dma_start(wo[:], w_out_t.rearrange("(o p) e -> p o e", p=P))

            qkt, vbuf, ot = {}, {}, {}

            def emit_proj_qk(b, j):
                ps = pg.tile([P, PL], F32, tag="g577", name="psqk")
                for k in range(DT):
                    for (c0, cw) in PCH:
                        nc.tensor.matmul(
                            ps[:, c0:c0 + cw],
                            lhsT=wqk[k][:, j * P:(j + 1) * P],
                            rhs=xt[(k, b)][:, c0:c0 + cw],
                            start=(k == 0), stop=(k == DT - 1),
                            skip_group_check=True)
                qt_tile = btp.tile([P, PL], FB, tag=f"qkt{j}", name=f"qkt{j}")
                nc.vector.tensor_scalar_add(qt_tile[:], ps[:], bqk[:, j:j + 1])
                qkt[(b, j)] = qt_tile

            def emit_proj_v(b, tt, pool=None):
                rows = QS[tt]
                ps = (pool or pg).tile([P, D], F32,
                                       tag="g577" if pool is None else "st",
                                       name="psv")
                for k in range(DT):
                    for (c0, cw) in VCH:
                        nc.tensor.matmul(
                            ps[:rows, c0:c0 + cw],
                            lhsT=xt[(k, b)][:, tt * P: tt * P + rows],
                            rhs=wvp[k][:, c0:c0 + cw],
                            start=(k == 0), stop=(k == DT - 1),
                            skip_group_check=True)
                vt = btp.tile([P, H, S + 1], FB, tag=f"v{tt}", name=f"v{tt}")
                nc.vector.memset(vt[:, :, S:S + 1], 1.0)
                nc.vector.tensor_copy(
                    vt[:rows, :, 0:S],
                    ps[:rows].rearrange("p (h s) -> p h s", h=H))
                vbuf[(b, tt)] = vt

            def emit_proj(b):
                # V first: its inputs (x(b0)+wv, interleaved DMAs) land first.
                # Alternate PSUM rings: the st ring is idle until attn(0), so
                # 4 V-chains can be in flight during the DMA feed.
                for tt in range(5):
                    emit_proj_v(b, tt, pool=pst if tt % 2 else None)
                for j in (0, 6, 1, 7, 2, 8, 3, 9, 4, 10, 5, 11):
                    emit_proj_qk(b, j)
                alloc_ot(b)

            def alloc_ot(b):
                for j in range(DT):
                    ot[(b, j)] = otp.tile([P, PL], FB, tag=f"ot{j}", name=f"ot{j}")

            def emit_av_norm(b, h, at, final=False):
                j2, hp = h // 2, (h % 2) * 64
                last = (b == NB - 1)
                # av shares the "st" ring (same shape); freed fast by the
                # copy-out so next-j2 STs resume quickly.
                av = pst.tile([P, PL], F32, tag="st", name="av")
                for qt in range(5):
                    rows = QS[qt]
                    for (c0, cw) in PCH:
                        nc.tensor.matmul(
                            av[0:S + 1, c0:c0 + cw],
                            lhsT=vbuf[(b, qt)][:rows, h, :],
                            rhs=at[:rows, qt, c0:c0 + cw],
                            start=(qt == 0), stop=(qt == 4),
                            skip_group_check=True)
                # copy PSUM->SBUF so the ring slot frees quickly. ACT has
                # slack in steady state; in the last batch (no proj work
                # left) ACT is the binding engine, so copy on DVE there.
                if final:
                    # very last chain: normalize straight from PSUM (the ring
                    # has no successors) — drops the copy from the critical
                    # path into the final out-projection.
                    rec = nrm.tile([1, PL], F32, tag="rec", name="rec")
                    nc.vector.reciprocal(rec[:], av[S:S + 1, :])
                    recb = nrm.tile([64, PL], F32, tag="recb", name="recb")
                    nc.gpsimd.partition_broadcast(recb[:], rec[:])
                    if hp == 0:
                        nc.vector.tensor_mul(
                            out=ot[(b, j2)][0:S, :], in0=av[0:S, :], in1=recb[:])
                    else:
                        tmp = nrm.tile([64, PL], FB, tag="tmp", name="tmp")
                        nc.vector.tensor_mul(out=tmp[:], in0=av[0:S, :],
                                             in1=recb[:])
                        nc.sync.dma_start(ot[(b, j2)][64:128, :], tmp[:])
                    return
                # copy on DVE: keeps ACT pure-Exp (a mixed ACT stream can
                # thrash activation-table loads on real hardware)
                avs = avsp.tile([S + 1, PL], F32, tag="avs", name="avs")
                nc.vector.tensor_copy(avs[:], av[0:S + 1, :])
                rec = nrm.tile([1, PL], F32, tag="rec", name="rec")
                nc.vector.reciprocal(rec[:], avs[S:S + 1, :])
                recb = nrm.tile([64, PL], F32, tag="recb", name="recb")
                nc.gpsimd.partition_broadcast(recb[:], rec[:])
                if hp == 0:
                    nc.vector.tensor_mul(
                        out=ot[(b, j2)][0:S, :], in0=avs[0:S, :], in1=recb[:])
                else:
                    tmp = nrm.tile([64, PL], FB, tag="tmp", name="tmp")
                    nc.vector.tensor_mul(out=tmp[:], in0=avs[0:S, :], in1=recb[:])
                    nc.sync.dma_start(ot[(b, j2)][64:128, :], tmp[:])

            def emit_attn_j2(b, j2):
                # head pairs in PE row halves (partitions 0-63 / 64-127):
                # their K=64 ST matmuls run in opposite PE row groups.
                h0 = 2 * j2
                qs_ = qkt[(b, j2)]
                ks_ = qkt[(b, 6 + j2)]
                at0 = atpool.tile([P, 5, PL], FB, tag="at", name="at0")
                at1 = atpool.tile([P, 5, PL], FB, tag="at", name="at1")
                for qt in range(5):
                    rows = QS[qt]
                    st0 = pst.tile([P, PL], F32, tag="st", name="st0")
                    st1 = pst.tile([P, PL], F32, tag="st", name="st1")
                    for (c0, cw) in PCH:
                        nc.tensor.matmul(
                            st0[:rows, c0:c0 + cw],
                            lhsT=ks_[0:64, qt * P: qt * P + rows],
                            rhs=qs_[0:64, c0:c0 + cw],
                            start=True, stop=True,
                            skip_group_check=True)
                        nc.tensor.matmul(
                            st1[:rows, c0:c0 + cw],
                            lhsT=ks_[64:128, qt * P: qt * P + rows],
                            rhs=qs_[64:128, c0:c0 + cw],
                            start=True, stop=True,
                            skip_group_check=True)
                    nc.scalar.activation(
                        at0[:rows, qt, :], st0[:rows, :],
                        mybir.ActivationFunctionType.Exp, scale=SCALE)
                    nc.scalar.activation(
                        at1[:rows, qt, :], st1[:rows, :],
                        mybir.ActivationFunctionType.Exp, scale=SCALE)
                fin = (b == NB - 1 and j2 == 5)
                if fin:
                    # odd head first: its SBUF->SBUF hop overlaps h0's chain
                    emit_av_norm(b, h0 + 1, at1, final=True)
                    emit_av_norm(b, h0, at0, final=True)
                else:
                    emit_av_norm(b, h0, at0)
                    emit_av_norm(b, h0 + 1, at1)

            def emit_outproj_m(b, m, pool=None, chop=False):
                ps = (pool or pg).tile([P, PL], F32,
                                       tag="g577" if pool is None else "st",
                                       name="psy")
                for k in range(DT):
                    for (c0, cw) in PCH:
                        nc.tensor.matmul(
                            ps[:, c0:c0 + cw],
                            lhsT=wo[:, k, m * P:(m + 1) * P],
                            rhs=ot[(b, k)][:, c0:c0 + cw],
                            start=(k == 0), stop=(k == DT - 1),
                            skip_group_check=True)
                ysb = ypool.tile([P, PL], FB, tag="ysb", name="ysb")
                if chop:
                    # drain overlap: ship the first 512 columns while the
                    # 65-col tail is still being summed
                    for (c0, cw) in PCH:
                        nc.vector.tensor_scalar_add(
                            ysb[:, c0:c0 + cw], ps[:, c0:c0 + cw],
                            bo[:, m:m + 1])
                        nc.sync.dma_start(
                            out_v[:, m, b * PL + c0: b * PL + c0 + cw],
                            ysb[:, c0:c0 + cw])
                else:
                    nc.vector.tensor_scalar_add(ysb[:], ps[:], bo[:, m:m + 1])
                    nc.sync.dma_start(out_v[:, m, b * PL:(b + 1) * PL], ysb[:])

            # Pipeline: per batch b, interleave attention j2-blocks with the
            # next batch's projection and the previous batch's out-projection
            # at fine grain, so the shared "g577" ring alternates short-hold
            # allocations and every engine always has runnable work nearby.
            # outproj chains are deferred ~two batches: they are the only
            # freely-schedulable PE work, so bank them as a backlog that
            # fills the ACT-bound final-batch window.
            emit_proj(0)
            for b in range(NB):
                for j2 in range(6):
                    emit_attn_j2(b, j2)
                    if b + 1 < NB:
                        emit_proj_qk(b + 1, 2 * j2)
                        emit_proj_qk(b + 1, 2 * j2 + 1)
                        if j2 < 5:
                            emit_proj_v(b + 1, j2)
                    if b == 2 and j2 % 2 == 0:
                        emit_outproj_m(0, j2 // 2)
                    if b == 3:
                        # backlog drain: outproj(0) tail, (1), (2) interleave
                        if j2 < 3:
                            emit_outproj_m(0, 3 + j2)
                        emit_outproj_m(1, j2)
                        emit_outproj_m(2, j2)
                if b + 1 < NB:
                    alloc_ot(b + 1)
            # final outproj: alternate PSUM rings (st ring is free now)
            for m in range(DT):
                emit_outproj_m(NB - 1, m, pool=pst if m % 2 else None,
                               chop=(m >= DT - 2))

    nc.compile()
    return nc


_NC = None


def _get_nc():
    global _NC
    if _NC is None:
        _NC = build_bass()
    return _NC


def make_in_maps(x, qkv_w, qkv_b, out_w, out_b):
    """Host-side shard + layout prep. Returns per-core input dicts."""
    bf16 = ml_dtypes.bfloat16
    x = np.asarray(x, dtype=np.float32)
    qkv_w = np.asarray(qkv_w, dtype=np.float32)
    qkv_b = np.asarray(qkv_b, dtype=np.float32)
    out_w = np.asarray(out_w, dtype=np.float32)
    out_b = np.asarray(out_b, dtype=np.float32)

    w_qkv_t = np.ascontiguousarray(qkv_w.T).astype(bf16)          # [768, 2304]
    w_out_t = np.ascontiguousarray(out_w.T).astype(bf16)          # [768, 768]
    b_qk = np.ascontiguousarray(qkv_b[D:3 * D].reshape(12, P).T)  # [128, 12]
    # v-bias passes linearly through the output projection (softmax rows sum
    # to 1): fold it into an effective output bias.
    b_eff = out_b + out_w @ qkv_b[0:D]
    b_out = np.ascontiguousarray(b_eff.reshape(DT, P).T)          # [128, 6]

    in_maps = []
    for c in range(NCORES):
        xc = x[c * NB:(c + 1) * NB].reshape(T, D)                 # [2308, 768]
        x_t = np.ascontiguousarray(xc.T).astype(bf16)             # [768, 2308]
        in_maps.append({
            "x_t": x_t,
            "w_qkv_t": w_qkv_t,
            "w_out_t": w_out_t,
            "b_qk": b_qk.astype(np.float32),
            "b_out": b_out.astype(np.float32),
        })
    return in_maps


def assemble_output(results):
    """Per-core 'out' [768, 2308] bf16 -> full [32, 577, 768] f32."""
    y = np.empty((B, PL, D), dtype=np.float32)
    for c in range(NCORES):
        yt = results[c]["out"].astype(np.float32)                 # [768, 2308]
        y[c * NB:(c + 1) * NB] = yt.T.reshape(NB, PL, D)
    return y


def run(x, qkv_w, qkv_b, out_w, out_b, trace=False):
    nc = _get_nc()
    in_maps = make_in_maps(x, qkv_w, qkv_b, out_w, out_b)
    res = run_bass_kernel_spmd(nc, in_maps, core_ids=list(range(NCORES)),
                               trace=trace)
    return assemble_output(res.results), res


def kernel(x, qkv_w, qkv_b, out_w, out_b):
    y, _ = run(x, qkv_w, qkv_b, out_w, out_b)
    return y


# revision 6
# speedup vs baseline: 2.6583x; 1.0861x over previous
r"""Trainium2 Bass kernel for nn_Attention (B=32, P=577, D=768, 12 heads).

Data-parallel over batch: 4 batch elements per core on 8 cores, zero
collectives. Transposed dataflow ([feature, token]) so every matmul
consumes the previous stage's output with no on-chip transposes:

  xT [768,577/b] --(wqkvT)--> qkT [1536,577] (features on partitions)
                         \--> V   [577,768]  (tokens on partitions)
  per (b,h): ST = K Q^T  (K=64 head pairs in PE row halves)
             AT = exp(scale*ST)  (ACT; |scale*S| small, no max-subtract)
  O_u^T[65,q] = [V|1]^T AT   (ones column -> softmax denominators row 64)
  DVE copies O_u^T PSUM->SBUF (frees the PSUM slot fast), then
  rec = 1/denom (DVE), bcast (Pool), O^T = O_u^T * rec (DVE)
  yT [768,577] = w_outT^T O^T + b_eff   (bf16 out DMA; host casts f32)

Schedule: per batch b, attention j2-blocks interleave at fine grain with
proj(b+1) and deferred outproj chains; outproj is deferred up to two
batches to bank PE work for the ACT-bound final-batch window.

PSUM (8 banks): ring "st" 2x[128,577] (4 banks) serves ST tiles AND av
accumulators; ring "g577" 2x[128,577] (4 banks) serves proj/outproj.
This keeps proj(b+1) PSUM independent of attn(b)'s normalization chain.

Engine discipline: ACT runs ONLY Exp (a mixed ACT stream risks
activation-table reloads on real HW, ~1.3us each); generic gpsimd tensor
ops avoided (Q7 launch/ucode cost on HW far exceeds the cost model);
Pool does only partition_broadcast.
"""

import numpy as np
import ml_dtypes

import concourse.bass as bass
import concourse.tile as tile
from concourse import bacc, mybir
from concourse.bass_utils import run_bass_kernel_spmd

B, PL, D = 32, 577, 768
H, S = 12, 64
NCORES = 8
NB = B // NCORES          # 4 batches per core
T = NB * PL               # 2308 tokens per core
P = 128
DT = D // P               # 6 contraction tiles
SCALE = float((D // 8) ** -0.5)   # 96**-0.5 (module bug kept faithful)

FB = mybir.dt.bfloat16
F32 = mybir.dt.float32

QS = [128, 128, 128, 128, 65]          # q-subtiles of 577
PCH = [(0, 512), (512, 65)]            # p-chunks of 577 (PSUM bank = 512 f32)
VCH = [(0, 512), (512, 256)]           # V projection chunks of 768


def build_bass(reps=1):
    nc = bacc.Bacc("TRN2", target_bir_lowering=False, debug=False,
                   num_devices=NCORES)

    x_t = nc.dram_tensor("x_t", [D, T], FB, kind="ExternalInput").ap()
    w_qkv_t = nc.dram_tensor("w_qkv_t", [D, 3 * D], FB, kind="ExternalInput").ap()
    w_out_t = nc.dram_tensor("w_out_t", [D, D], FB, kind="ExternalInput").ap()
    b_qk = nc.dram_tensor("b_qk", [P, 12], F32, kind="ExternalInput").ap()
    b_out = nc.dram_tensor("b_out", [P, DT], F32, kind="ExternalInput").ap()
    out_d = nc.dram_tensor("out", [D, T], FB, kind="ExternalOutput").ap()
    out_v = out_d.rearrange("(o p) t -> p o t", p=P)
    xv = x_t.rearrange("(o p) t -> p o t", p=P)
    wv = w_qkv_t.rearrange("(o p) e -> p o e", p=P)

    with tile.TileContext(nc) as tc:
      for _rep in range(reps):  # >1 only for differential benchmarking
        with tc.tile_pool(name="singles", bufs=1) as singles, \
             tc.tile_pool(name="bt", bufs=2) as btp, \
             tc.tile_pool(name="otp", bufs=4) as otp, \
             tc.tile_pool(name="atp", bufs=4) as atpool, \
             tc.tile_pool(name="nrm", bufs=6) as nrm, \
             tc.tile_pool(name="avs", bufs=4) as avsp, \
             tc.tile_pool(name="yout", bufs=6) as ypool, \
             tc.tile_pool(name="pst", bufs=2, space="PSUM") as pst, \
             tc.tile_pool(name="pg", bufs=2, space="PSUM") as pg:
            # ---- input DMAs: interleave x(b0) and wv k-tiles so the V
            # projection (cheapest full dependency set) starts ~2us in; then
            # wqk for the QK chains; then the rest of x merged per k.
            xt = {}
            wvp, wqk = [], []
            for k in range(DT):
                xk = singles.tile([P, PL], FB, tag=f"x{k}_0", name=f"x{k}_0")
                nc.sync.dma_start(xk[:], xv[:, k, 0:PL])
                xt[(k, 0)] = xk
                wk = singles.tile([P, D], FB, tag=f"wv{k}", name=f"wv{k}")
                nc.sync.dma_start(wk[:], wv[:, k, 0:D])
                wvp.append(wk)
            for k in range(DT):
                wk = singles.tile([P, 2 * D], FB, tag=f"wqk{k}",
                                  name=f"wqk{k}")
                nc.sync.dma_start(wk[:], wv[:, k, D:3 * D])
                wqk.append(wk)
            bqk = singles.tile([P, 12], F32, tag="bqk")
            nc.sync.dma_start(bqk[:], b_qk)
            for k in range(DT):
                xk = singles.tile([P, (NB - 1) * PL], FB, tag=f"x{k}_r",
                                  name=f"x{k}_r")
                nc.sync.dma_start(xk[:], xv[:, k, PL:T])
                for b in range(1, NB):
                    xt[(k, b)] = xk[:, (b - 1) * PL: b * PL]
            bo = singles.tile([P, DT], F32, tag="bo")
            nc.sync.dma_start(bo[:], b_out)
            wo = singles.tile([P, DT, D], FB, tag="wo")
            nc.sync.dma_start(wo[:], w_out_t.rearrange("(o p) e -> p o e", p=P))

            qkt, vbuf, ot = {}, {}, {}

            def emit_proj_qk(b, j):
                ps = pg.tile([P, PL], F32, tag="g577", name="psqk")
                for k in range(DT):
                    for (c0, cw) in PCH:
                        nc.tensor.matmul(
                            ps[:, c0:c0 + cw],
                            lhsT=wqk[k][:, j * P:(j + 1) * P],
                            rhs=xt[(k, b)][:, c0:c0 + cw],
                            start=(k == 0), stop=(k == DT - 1),
                            skip_group_check=True)
                qt_tile = btp.tile([P, PL], FB, tag=f"qkt{j}", name=f"qkt{j}")
                nc.vector.tensor_scalar_add(qt_tile[:], ps[:], bqk[:, j:j + 1])
                qkt[(b, j)] = qt_tile

            def emit_proj_v(b, tt, pool=None):
                rows = QS[tt]
                ps = (pool or pg).tile([P, D], F32,
                                       tag="g577" if pool is None else "st",
                                       name="psv")
                for k in range(DT):
                    for (c0, cw) in VCH:
                        nc.tensor.matmul(
                            ps[:rows, c0:c0 + cw],
                            lhsT=xt[(k, b)][:, tt * P: tt * P + rows],
                            rhs=wvp[k][:, c0:c0 + cw],
                            start=(k == 0), stop=(k == DT - 1),
                            skip_group_check=True)
                vt = btp.tile([P, H, S + 1], FB, tag=f"v{tt}", name=f"v{tt}")
                nc.vector.memset(vt[:, :, S:S + 1], 1.0)
                nc.vector.tensor_copy(
                    vt[:rows, :, 0:S],
                    ps[:rows].rearrange("p (h s) -> p h s", h=H))
                vbuf[(b, tt)] = vt

            def emit_proj(b):
                # V first: its inputs (x(b0)+wv, interleaved DMAs) land first.
                # Alternate PSUM rings: the st ring is idle until attn(0), so
                # 4 V-chains can be in flight during the DMA feed.
                for tt in range(5):
                    emit_proj_v(b, tt, pool=pst if tt % 2 else None)
                for j in (0, 6, 1, 7, 2, 8, 3, 9, 4, 10, 5, 11):
                    emit_proj_qk(b, j)
                alloc_ot(b)

            def alloc_ot(b):
                for j in range(DT):
                    ot[(b, j)] = otp.tile([P, PL], FB, tag=f"ot{j}", name=f"ot{j}")

            def emit_av_norm(b, h, at, final=False):
                j2, hp = h // 2, (h % 2) * 64
                last = (b == NB - 1)
                # av shares the "st" ring (same shape); freed fast by the
                # copy-out so next-j2 STs resume quickly.
                av = pst.tile([P, PL], F32, tag="st", name="av")
                for qt in range(5):
                    rows = QS[qt]
                    for (c0, cw) in PCH:
                        nc.tensor.matmul(
                            av[0:S + 1, c0:c0 + cw],
                            lhsT=vbuf[(b, qt)][:rows, h, :],
                            rhs=at[:rows, qt, c0:c0 + cw],
                            start=(qt == 0), stop=(qt == 4),
                            skip_group_check=True)
                # copy PSUM->SBUF so the ring slot frees quickly. ACT has
                # slack in steady state; in the last batch (no proj work
                # left) ACT is the binding engine, so copy on DVE there.
                if final:
                    # very last chain: normalize straight from PSUM (the ring
                    # has no successors) — drops the copy from the critical
                    # path into the final out-projection.
                    rec = nrm.tile([1, PL], F32, tag="rec", name="rec")
                    nc.vector.reciprocal(rec[:], av[S:S + 1, :])
                    recb = nrm.tile([64, PL], F32, tag="recb", name="recb")
                    nc.gpsimd.partition_broadcast(recb[:], rec[:])
                    if hp == 0:
                        nc.vector.tensor_mul(
                            out=ot[(b, j2)][0:S, :], in0=av[0:S, :], in1=recb[:])
                    else:
                        tmp = nrm.tile([64, PL], FB, tag="tmp", name="tmp")
                        nc.vector.tensor_mul(out=tmp[:], in0=av[0:S, :],
                                             in1=recb[:])
                        nc.sync.dma_start(ot[(b, j2)][64:128, :], tmp[:])
                    return
                # copy on DVE: keeps ACT pure-Exp (a mixed ACT stream can
                # thrash activation-table loads on real hardware)
                avs = avsp.tile([S + 1, PL], F32, tag="avs", name="avs")
                nc.vector.tensor_copy(avs[:], av[0:S + 1, :])
                rec = nrm.tile([1, PL], F32, tag="rec", name="rec")
                nc.vector.reciprocal(rec[:], avs[S:S + 1, :])
                recb = nrm.tile([64, PL], F32, tag="recb", name="recb")
                nc.gpsimd.partition_broadcast(recb[:], rec[:])
                if hp == 0:
                    nc.vector.tensor_mul(
                        out=ot[(b, j2)][0:S, :], in0=avs[0:S, :], in1=recb[:])
                else:
                    tmp = nrm.tile([64, PL], FB, tag="tmp", name="tmp")
                    nc.vector.tensor_mul(out=tmp[:], in0=avs[0:S, :], in1=recb[:])
                    nc.sync.dma_start(ot[(b, j2)][64:128, :], tmp[:])

            def emit_attn_j2(b, j2):
                # head pairs in PE row halves (partitions 0-63 / 64-127):
                # their K=64 ST matmuls run in opposite PE row groups.
                h0 = 2 * j2
                qs_ = qkt[(b, j2)]
                ks_ = qkt[(b, 6 + j2)]
                at0 = atpool.tile([P, 5, PL], FB, tag="at", name="at0")
                at1 = atpool.tile([P, 5, PL], FB, tag="at", name="at1")
                for qt in range(5):
                    rows = QS[qt]
                    st0 = pst.tile([P, PL], F32, tag="st", name="st0")
                    st1 = pst.tile([P, PL], F32, tag="st", name="st1")
                    for (c0, cw) in PCH:
                        nc.tensor.matmul(
                            st0[:rows, c0:c0 + cw],
                            lhsT=ks_[0:64, qt * P: qt * P + rows],
                            rhs=qs_[0:64, c0:c0 + cw],
                            start=True, stop=True,
                            skip_group_check=True)
                        nc.tensor.matmul(
                            st1[:rows, c0:c0 + cw],
                            lhsT=ks_[64:128, qt * P: qt * P + rows],
                            rhs=qs_[64:128, c0:c0 + cw],
                            start=True, stop=True,
                            skip_group_check=True)
                    nc.scalar.activation(
                        at0[:rows, qt, :], st0[:rows, :],
                        mybir.ActivationFunctionType.Exp, scale=SCALE)
                    nc.scalar.activation(
                        at1[:rows, qt, :], st1[:rows, :],
                        mybir.ActivationFunctionType.Exp, scale=SCALE)
                fin = (b == NB - 1 and j2 == 5)
                if fin:
                    # odd head first: its SBUF->SBUF hop overlaps h0's chain
                    emit_av_norm(b, h0 + 1, at1, final=True)
                    emit_av_norm(b, h0, at0, final=True)
                else:
                    emit_av_norm(b, h0, at0)
                    emit_av_norm(b, h0 + 1, at1)

            def emit_outproj_m(b, m, pool=None, chop=False):
                ps = (pool or pg).tile([P, PL], F32,
                                       tag="g577" if pool is None else "st",
                                       name="psy")
                for k in range(DT):
                    for (c0, cw) in PCH:
                        nc.tensor.matmul(
                            ps[:, c0:c0 + cw],
                            lhsT=wo[:, k, m * P:(m + 1) * P],
                            rhs=ot[(b, k)][:, c0:c0 + cw],
                            start=(k == 0), stop=(k == DT - 1),
                            skip_group_check=True)
                ysb = ypool.tile([P, PL], FB, tag="ysb", name="ysb")
                if chop:
                    # drain overlap: ship the first 512 columns while the
                    # 65-col tail is still being summed
                    for (c0, cw) in PCH:
                        nc.vector.tensor_scalar_add(
                            ysb[:, c0:c0 + cw], ps[:, c0:c0 + cw],
                            bo[:, m:m + 1])
                        nc.sync.dma_start(
                            out_v[:, m, b * PL + c0: b * PL + c0 + cw],
                            ysb[:, c0:c0 + cw])
                else:
                    nc.vector.tensor_scalar_add(ysb[:], ps[:], bo[:, m:m + 1])
                    nc.sync.dma_start(out_v[:, m, b * PL:(b + 1) * PL], ysb[:])

            # Pipeline: per batch b, interleave attention j2-blocks with the
            # next batch's projection and the previous batch's out-projection
            # at fine grain, so the shared "g577" ring alternates short-hold
            # allocations and every engine always has runnable work nearby.
            # outproj chains are deferred ~two batches: they are the only
            # freely-schedulable PE work, so bank them as a backlog that
            # fills the ACT-bound final-batch window.
            emit_proj(0)
            for b in range(NB):
                for j2 in range(6):
                    emit_attn_j2(b, j2)
                    if b + 1 < NB:
                        emit_proj_qk(b + 1, 2 * j2)
                        emit_proj_qk(b + 1, 2 * j2 + 1)
                        if j2 < 5:
                            emit_proj_v(b + 1, j2)
                    if b == 2 and j2 % 2 == 0:
                        emit_outproj_m(0, j2 // 2)
                    if b == 3:
                        # backlog drain: outproj(0) tail, (1), (2) interleave
                        if j2 < 3:
                            emit_outproj_m(0, 3 + j2)
                        emit_outproj_m(1, j2)
                        emit_outproj_m(2, j2)
                if b + 1 < NB:
                    alloc_ot(b + 1)
            # final outproj: alternate PSUM rings (st ring is free now)
            for m in range(DT):
                emit_outproj_m(NB - 1, m, pool=pst if m % 2 else None,
                               chop=(m >= DT - 2))

    nc.compile()
    return nc


_NC = None


def _get_nc():
    global _NC
    if _NC is None:
        _NC = build_bass()
    return _NC


def make_in_maps(x, qkv_w, qkv_b, out_w, out_b):
    """Host-side shard + layout prep. Returns per-core input dicts."""
    bf16 = ml_dtypes.bfloat16
    x = np.asarray(x, dtype=np.float32)
    qkv_w = np.asarray(qkv_w, dtype=np.float32)
    qkv_b = np.asarray(qkv_b, dtype=np.float32)
    out_w = np.asarray(out_w, dtype=np.float32)
    out_b = np.asarray(out_b, dtype=np.float32)

    w_qkv_t = np.ascontiguousarray(qkv_w.T).astype(bf16)          # [768, 2304]
    w_out_t = np.ascontiguousarray(out_w.T).astype(bf16)          # [768, 768]
    b_qk = np.ascontiguousarray(qkv_b[D:3 * D].reshape(12, P).T)  # [128, 12]
    # v-bias passes linearly through the output projection (softmax rows sum
    # to 1): fold it into an effective output bias.
    b_eff = out_b + out_w @ qkv_b[0:D]
    b_out = np.ascontiguousarray(b_eff.reshape(DT, P).T)          # [128, 6]

    in_maps = []
    for c in range(NCORES):
        xc = x[c * NB:(c + 1) * NB].reshape(T, D)                 # [2308, 768]
        x_t = np.ascontiguousarray(xc.T).astype(bf16)             # [768, 2308]
        in_maps.append({
            "x_t": x_t,
            "w_qkv_t": w_qkv_t,
            "w_out_t": w_out_t,
            "b_qk": b_qk.astype(np.float32),
            "b_out": b_out.astype(np.float32),
        })
    return in_maps


def assemble_output(results):
    """Per-core 'out' [768, 2308] bf16 -> full [32, 577, 768] f32."""
    y = np.empty((B, PL, D), dtype=np.float32)
    for c in range(NCORES):
        yt = results[c]["out"].astype(np.float32)                 # [768, 2308]
        y[c * NB:(c + 1) * NB] = yt.T.reshape(NB, PL, D)
    return y


def run(x, qkv_w, qkv_b, out_w, out_b, trace=False):
    nc = _get_nc()
    in_maps = make_in_maps(x, qkv_w, qkv_b, out_w, out_b)
    res = run_bass_kernel_spmd(nc, in_maps, core_ids=list(range(NCORES)),
                               trace=trace)
    return assemble_output(res.results), res


def kernel(x, qkv_w, qkv_b, out_w, out_b):
    y, _ = run(x, qkv_w, qkv_b, out_w, out_b)
    return y
